# revision 1
# baseline (speedup 1.0000x reference)
"""Trainium2 Bass kernel: GPT-style transformer forward pass.

Strategy: data-parallel over batch across 8 NeuronCores (B=64 -> 8 per core),
weights replicated.  On each core, activations are kept feature-major
(x_T [D=384 (3x128 partitions), 2048 tokens]) so matmuls contract the
partition dim with no activation transposes; LN/QKV/proj/FFN run on
512-token pairs of batch elements.  All matmuls are float32r (full rate at
moving dim >= 256).  LayerNorm stats come from ones-vector matmuls
(partition reduction); rstd = exp(-0.5*ln(var+eps)) on the Scalar engine so
the whole kernel lives in the natural_log_exp activation table set (no
table switches).  Attention computes transposed scores S_T[s,t] per (b,h),
multiplicative causal mask after exp, then token-major PV with an appended
ones-column in V so softmax denominators land as a per-partition column
(cheap [128,6] reciprocal + one broadcast multiply), and PE transposes
carry att back to feature-major for the projection.
"""

import os
import sys

for _p in ("/opt/trn_rl_repo",):
    if _p not in sys.path and os.path.isdir(_p):
        sys.path.insert(0, _p)

import numpy as np

import concourse.bass as bass
import concourse.mybir as mybir
import concourse.tile as tile
from concourse import bacc
from concourse.bass_utils import run_bass_kernel_spmd

F32 = mybir.dt.float32
F32R = mybir.dt.float32r
AF = mybir.ActivationFunctionType
OP = mybir.AluOpType

V, D, H, HD, L, T, B = 65, 384, 6, 64, 6, 256, 64
NCORES = 8
BL = B // NCORES          # batch elements per core
NTOK = BL * T             # tokens per core
DFF = 4 * D               # 1536
EPS = 1e-3
KC = D // 128             # 3 contraction chunks of 128
MT = D // 128             # 3 output feature tiles
FT = DFF // 128           # 12 ffn tiles
HD1 = HD + 2              # V columns incl ones col (64) + pad (65)

USE_F32R = os.environ.get("KERNEL_NO_F32R", "") != "1"
MDT = F32R if USE_F32R else F32

# cst layout: cols 0:512 ones, col 512 = 1/D, cols 513:641 identity 128x128,
# cols 641:769 = 1/D block (stats matmul lhsT)
CST_W = 513 + 128 + 128


def _patch_act_tables():
    """Steer the activation-table picker to natural_log_exp_and_others for
    Exp and Ln, so this kernel's ACT stream never switches table sets.
    Set names/indices are preserved (walrus resolves the same act_info),
    only the picker's view of which sets provide Exp/Ln is narrowed."""
    if getattr(bacc, "_act_tables_patched", False):
        return
    real = bacc.get_activation_tables

    def patched(arch):
        t = real(arch)
        exp = mybir.ActivationFunctionType.Exp
        ln = mybir.ActivationFunctionType.Ln
        out = {}
        for name, fns in t.items():
            if name != "natural_log_exp_and_others":
                fns = fns - {exp, ln}
            out[name] = fns
        return out

    bacc.get_activation_tables = patched
    bacc._act_tables_patched = True


class _MM:
    """matmul emitter with explicit accumulation-chain boundaries."""

    def __init__(self, nc):
        self.nc = nc

    def __call__(self, out, lhsT, rhs, first=True, last=True, tile_position=None):
        self.nc.tensor.matmul(
            out, lhsT, rhs, start=first, stop=last, tile_position=tile_position,
        )


def build_program(n_layers=L, n_b=BL, n_heads=H):
    _patch_act_tables()
    assert n_b % 2 == 0 or n_b == 1
    ntok = n_b * T
    npair = max(1, n_b // 2)
    PW = 512 if n_b > 1 else 256      # tokens per pair-chunk
    nc = bacc.Bacc("TRN2", target_bir_lowering=False, debug=False)

    # ---------------- DRAM I/O ----------------
    hot_d = nc.dram_tensor("hotT", [V, ntok], MDT, kind="ExternalInput").ap()
    temb_d = nc.dram_tensor("temb", [V, D], MDT, kind="ExternalInput").ap()
    pos_d = nc.dram_tensor("posT", [128, KC, 512], F32, kind="ExternalInput").ap()
    mask_d = nc.dram_tensor("mask", [128, 512], F32, kind="ExternalInput").ap()
    wqkv_d = nc.dram_tensor("wqkv", [n_layers, 128, 3, KC, D], MDT, kind="ExternalInput").ap()
    wproj_d = nc.dram_tensor("wproj", [n_layers, 128, KC, D], MDT, kind="ExternalInput").ap()
    w1_d = nc.dram_tensor("w1", [n_layers, 128, KC, DFF], MDT, kind="ExternalInput").ap()
    w2_d = nc.dram_tensor("w2", [n_layers, 128, FT, D], MDT, kind="ExternalInput").ap()
    vbias_d = nc.dram_tensor("vbias", [n_layers, 128, D], F32, kind="ExternalInput").ap()
    biasc_d = nc.dram_tensor("biasc", [n_layers, 128, MT + FT + MT], F32, kind="ExternalInput").ap()
    biasr_d = nc.dram_tensor("biasr", [n_layers, 1, 2 * D], MDT, kind="ExternalInput").ap()
    whead_d = nc.dram_tensor("whead", [128, KC, V], MDT, kind="ExternalInput").ap()
    bhead_d = nc.dram_tensor("bheadc", [V, 1], F32, kind="ExternalInput").ap()
    cst_d = nc.dram_tensor("cst", [128, CST_W], MDT, kind="ExternalInput").ap()
    out_d = nc.dram_tensor("logitsT", [n_b, V, T], F32, kind="ExternalOutput").ap()

    from contextlib import ExitStack

    with tile.TileContext(nc) as tc, \
         nc.allow_low_precision(reason="fp32r matmul operand production"), \
         ExitStack() as ctx:
        ep = ctx.enter_context

        # ---------------- pools ----------------
        cpool = ep(tc.tile_pool(name="consts", bufs=1))
        xpool = ep(tc.tile_pool(name="x", bufs=1))
        wpool_qkv = ep(tc.tile_pool(name="wqkv", bufs=1))
        wpool_proj = ep(tc.tile_pool(name="wproj", bufs=1))
        wpool_1 = ep(tc.tile_pool(name="w1", bufs=1))
        wpool_2 = ep(tc.tile_pool(name="w2", bufs=1))
        wpool_b = ep(tc.tile_pool(name="wbias", bufs=1))
        hpool = ep(tc.tile_pool(name="h", bufs=3))
        xsqpool = ep(tc.tile_pool(name="xsq", bufs=1))
        qpool = ep(tc.tile_pool(name="q", bufs=2))
        kpool = ep(tc.tile_pool(name="k", bufs=2))
        vpool = ep(tc.tile_pool(name="v", bufs=1))
        upool = ep(tc.tile_pool(name="u", bufs=4))
        atmpool = ep(tc.tile_pool(name="atm", bufs=2))
        attpool = ep(tc.tile_pool(name="att", bufs=1))
        h1pool = ep(tc.tile_pool(name="h1", bufs=1))
        lgpool = ep(tc.tile_pool(name="lg", bufs=1))
        stpool = ep(tc.tile_pool(name="st", bufs=6))
        rdpool = ep(tc.tile_pool(name="rd", bufs=4))

        pbig = ep(tc.tile_pool(name="pbig", bufs=4, space="PSUM"))
        ppv = ep(tc.tile_pool(name="ppv", bufs=2, space="PSUM"))
        pstat = ep(tc.tile_pool(name="pstat", bufs=2, space="PSUM"))

        mm = _MM(nc)

        # ---------------- constants ----------------
        cst = cpool.tile([128, CST_W], MDT, name="cst_c")
        nc.sync.dma_start(out=cst[:, :], in_=cst_d[:, :])
        ones = cst[:, 0:512]
        ident = cst[:, 513:641]
        invD = cst[:, 641:769]
        mask = cpool.tile([128, 512], F32, name="mask_c")
        nc.sync.dma_start(out=mask[:, :], in_=mask_d[:, :])
        whead = cpool.tile([128, KC, V], MDT, name="whead_c")
        nc.sync.dma_start(out=whead[:, :, :], in_=whead_d[:, :, :])
        bhead = cpool.tile([V, 1], F32, name="bhead_c")
        nc.sync.dma_start(out=bhead[:, :], in_=bhead_d[:, :])

        x = xpool.tile([128, KC, ntok], MDT, name="x_resid")

        nbp = PW // T
        v_tiles = [vpool.tile([128, 2 * nbp, n_heads, HD1], MDT, name=f"v_pp{i}")
                   for i in range(2)]
        for vt in v_tiles:
            for tb in range(2 * nbp):
                nc.vector.tensor_copy(vt[:, tb, :, HD:HD1],
                                      ones[:, 0:2 * n_heads].rearrange(
                                          "p (h c) -> p h c", h=n_heads))

        # ---------------- embedding (scratch tiles borrow other pools) ----------------
        pos = attpool.tile([128, KC, 512], F32, tag="att")
        nc.sync.dma_start(out=pos[:, :, :], in_=pos_d[:, :, :])
        temb = stpool.tile([V, 384], MDT, tag="st")
        nc.sync.dma_start(out=temb[:, :], in_=temb_d[:, :])
        for ch in range(ntok // PW):
            cs = slice(ch * PW, ch * PW + PW)
            hot = xsqpool.tile([V, PW], MDT, tag="xsq")
            nc.sync.dma_start(out=hot[:, :], in_=hot_d[:, cs])
            for c in range(KC):
                ps = pbig.tile([128, 512], F32, tag="pbig")
                mm(ps[:, 0:PW], temb[0:V, c * 128:(c + 1) * 128], hot[0:V, :])
                nc.vector.tensor_tensor(x[:, c, cs], ps[:, 0:PW], pos[:, c, 0:PW], op=OP.add)

        # ---------------- LN split into stats + finish ----------------
        def ln_stats(p):
            """square + M=128 stat matmuls (mean/meansq arrive pre-broadcast
            across partitions) + the DVE/ACT rstd chain, all off the PE"""
            pc = slice(p * PW, p * PW + PW)
            xsq = xsqpool.tile([128, KC, PW], MDT, tag="xsq")
            nc.scalar.activation(xsq[:, :, :], x[:, :, pc], AF.Square)
            meanb = pstat.tile([128, PW], F32, tag="pstat")
            msqb = pstat.tile([128, PW], F32, tag="pstat")
            for c in range(KC):
                mm(meanb[:, :], invD[:, :], x[:, c, pc],
                   first=(c == 0), last=(c == KC - 1))
            for c in range(KC):
                mm(msqb[:, :], invD[:, :], xsq[:, c, :],
                   first=(c == 0), last=(c == KC - 1))
            m2b = stpool.tile([128, PW], F32, tag="st")
            nc.scalar.activation(m2b[:, :], meanb[:, :], AF.Square)
            varb = stpool.tile([128, PW], F32, tag="st")
            nc.vector.scalar_tensor_tensor(
                varb[:, :], msqb[:, :], EPS, m2b[:, :],
                op0=OP.add, op1=OP.subtract)
            nc.scalar.activation(varb[:, :], varb[:, :], AF.Ln)
            rstdb = stpool.tile([128, PW], MDT, tag="st")
            nc.scalar.activation(rstdb[:, :], varb[:, :], AF.Exp, scale=-0.5)
            mrb = stpool.tile([128, PW], MDT, tag="st")
            nc.vector.tensor_tensor(mrb[:, :], meanb[:, :], rstdb[:, :], op=OP.mult)
            return (p, rstdb, mrb)

        def ln_finish(tok):
            """apply -> h (pure DVE; no PE work)"""
            p, rstdb, mrb = tok
            pc = slice(p * PW, p * PW + PW)
            h = hpool.tile([128, KC, PW], MDT, tag="h")
            for c in range(KC):
                nc.vector.tensor_tensor(h[:, c, :], x[:, c, pc], rstdb[:, :], op=OP.mult)
                nc.vector.tensor_tensor(h[:, c, :], h[:, c, :], mrb[:, :], op=OP.subtract)
            return h

        def run(mids):
            out = []
            for f in mids:
                out.append(f())
            return out

        # ---------------- phase B: qkv + attention + proj for a pair ----------------
        def emit_B(p, h, wqkv, wproj, vbias, biasc, biasr, mid_a=(), mid_b=()):
            pc = slice(p * PW, p * PW + PW)
            nb_in_p = PW // T
            q_t = qpool.tile([128, MT, PW], MDT, tag="q")
            k_t = kpool.tile([128, MT, PW], MDT, tag="k")
            for mat, dst in ((0, q_t), (1, k_t)):
                for mt in range(MT):
                    ps = pbig.tile([128, 512], F32, tag="pbig")
                    for kc in range(KC):
                        mm(ps[:, 0:PW], wqkv[:, mat, kc, mt * 128:(mt + 1) * 128],
                           h[:, kc, :], first=(kc == 0), last=False)
                    mm(ps[:, 0:PW],
                       biasr[0:1, mat * D + mt * 128: mat * D + (mt + 1) * 128],
                       ones[0:1, 0:PW], first=False, last=True)
                    nc.vector.tensor_copy(dst[:, mt, :], ps[:, 0:PW])
            v_t = v_tiles[p % 2]
            for tb in range(2 * nb_in_p):
                vps = pbig.tile([128, 512], F32, tag="pbig")
                for kc in range(KC):
                    mm(vps[:, 0:D], h[:, kc, tb * 128:(tb + 1) * 128],
                       wqkv[:, 2, kc, :], first=(kc == 0), last=(kc == KC - 1))
                nc.vector.tensor_tensor(
                    v_t[:, tb, :, 0:HD],
                    vps[:, 0:D].rearrange("p (h d) -> p h d", h=n_heads),
                    vbias[:, :].rearrange("p (h d) -> p h d", h=n_heads),
                    op=OP.add)
            mids_out = run(mid_a)
            atms = []
            for bi in range(nb_in_p):
                boff = bi * T
                if bi == 1:
                    mids_out += run(mid_b)
                us = [None] * n_heads
                pv0 = ppv.tile([128, n_heads, HD1], F32, tag="ppv")
                pv1 = ppv.tile([128, n_heads, HD1], F32, tag="ppv")

                def emit_S(hh):
                    hp = 64 * (hh % 2)
                    hc = hh // 2
                    sps = pbig.tile([128, 512], F32, tag="pbig")
                    mm(sps[:, 0:256], k_t[hp:hp + HD, hc, boff:boff + 128],
                       q_t[hp:hp + HD, hc, boff:boff + T])
                    mm(sps[:, 256:512], k_t[hp:hp + HD, hc, boff + 128:boff + 256],
                       q_t[hp:hp + HD, hc, boff:boff + T])
                    u_t = upool.tile([128, 512], MDT, tag="u")
                    nc.scalar.activation(u_t[:, :], sps[:, :], AF.Exp)
                    nc.vector.tensor_tensor(u_t[:, :], u_t[:, :], mask[:, :], op=OP.mult)
                    us[hh] = u_t

                def emit_PV(hh):
                    u_t = us[hh]
                    mm(pv0[:, hh, :], u_t[:, 0:128], v_t[:, 2 * bi, hh, :])
                    mm(pv1[:, hh, :], u_t[:, 128:256], v_t[:, 2 * bi, hh, :],
                       first=True, last=False)
                    mm(pv1[:, hh, :], u_t[:, 384:512], v_t[:, 2 * bi + 1, hh, :],
                       first=False, last=True)

                # window the S/PV interleave so only ~3 U tiles are live
                emit_S(0); emit_S(1); emit_S(2)
                emit_PV(0); emit_S(3)
                emit_PV(1); emit_S(4)
                emit_PV(2); emit_S(5)
                emit_PV(3); emit_PV(4); emit_PV(5)
                atm = atmpool.tile([128, 2, n_heads * HD], MDT, tag="atm")
                for tb, pv in ((0, pv0), (1, pv1)):
                    rden = rdpool.tile([128, n_heads], F32, tag="rd")
                    nc.vector.reciprocal(rden[:, :], pv[:, :, HD])
                    nc.vector.tensor_tensor(
                        atm[:, tb, :].rearrange("p (h d) -> p h d", h=n_heads),
                        pv[:, :, 0:HD],
                        rden[:, :, None].broadcast_to([128, n_heads, HD]),
                        op=OP.mult)
                atms.append(atm)
            att_t = attpool.tile([128, KC, PW], MDT, tag="att")
            for c in range(KC):
                tps = pbig.tile([128, 512], MDT, tag="pbig")
                for bi in range(nb_in_p):
                    for tb in range(2):
                        col = (bi * 2 + tb) * 128
                        nc.tensor.transpose(
                            tps[:, col:col + 128],
                            atms[bi][:, tb, c * 128:(c + 1) * 128],
                            ident[:, :])
                nc.vector.tensor_copy(att_t[:, c, :], tps[:, 0:PW])
            for mt in range(MT):
                pp = pbig.tile([128, 512], F32, tag="pbig")
                for kc in range(KC):
                    mm(pp[:, 0:PW], wproj[:, kc, mt * 128:(mt + 1) * 128],
                       att_t[:, kc, :], first=(kc == 0), last=(kc == KC - 1))
                nc.vector.scalar_tensor_tensor(
                    x[:, mt, pc], pp[:, 0:PW], biasc[:, mt:mt + 1], x[:, mt, pc],
                    op0=OP.add, op1=OP.add)
            return mids_out

        # ---------------- phase D: FFN for a pair ----------------
        def emit_D(p, h2, w1, w2, biasc, mid=()):
            pc = slice(p * PW, p * PW + PW)
            h1_t = h1pool.tile([128, FT, PW], MDT, tag="h1")
            for mt in range(FT):
                fps = pbig.tile([128, 512], F32, tag="pbig")
                for kc in range(KC):
                    mm(fps[:, 0:PW], w1[:, kc, mt * 128:(mt + 1) * 128],
                       h2[:, kc, :], first=(kc == 0), last=(kc == KC - 1))
                nc.scalar.activation(h1_t[:, mt, :], fps[:, 0:PW], AF.Relu,
                                     bias=biasc[:, MT + mt:MT + mt + 1])
            mids_out = run(mid)
            for mt in range(MT):
                fp2 = pbig.tile([128, 512], F32, tag="pbig")
                for kc in range(FT):
                    mm(fp2[:, 0:PW], w2[:, kc, mt * 128:(mt + 1) * 128],
                       h1_t[:, kc, :], first=(kc == 0), last=(kc == FT - 1))
                nc.vector.scalar_tensor_tensor(
                    x[:, mt, pc], fp2[:, 0:PW],
                    biasc[:, MT + FT + mt:MT + FT + mt + 1],
                    x[:, mt, pc], op0=OP.add, op1=OP.add)
            return mids_out

        # ---------------- layers: software-pipelined emission ----------------
        carry = {}
        for l in range(n_layers):
            wqkv = wpool_qkv.tile([128, 3, KC, D], MDT, tag="wqkv")
            nc.sync.dma_start(out=wqkv[:, :, :, :], in_=wqkv_d[l])
            wproj = wpool_proj.tile([128, KC, D], MDT, tag="wproj")
            nc.sync.dma_start(out=wproj[:, :, :], in_=wproj_d[l])
            w1 = wpool_1.tile([128, KC, DFF], MDT, tag="w1")
            nc.sync.dma_start(out=w1[:, :, :], in_=w1_d[l])
            w2 = wpool_2.tile([128, FT, D], MDT, tag="w2")
            nc.sync.dma_start(out=w2[:, :, :], in_=w2_d[l])
            vbias = wpool_b.tile([128, D], F32, tag="vbias")
            nc.sync.dma_start(out=vbias[:, :], in_=vbias_d[l])
            biasc = wpool_b.tile([128, MT + FT + MT], F32, tag="biasc")
            nc.sync.dma_start(out=biasc[:, :], in_=biasc_d[l])
            biasr = wpool_b.tile([1, 2 * D], MDT, tag="biasr")
            nc.sync.dma_start(out=biasr[0:1, :], in_=biasr_d[l])

            B = lambda p, h, **kw: emit_B(p, h, wqkv, wproj, vbias, biasc, biasr, **kw)
            Dp = lambda p, h2, **kw: emit_D(p, h2, w1, w2, biasc, **kw)

            if npair == 4:
                if l == 0:
                    h0 = ln_finish(ln_stats(0))
                    s1 = ln_stats(1)
                else:
                    h0, s1 = carry["h0"], carry["s1"]
                (h1,) = B(0, h0, mid_a=[lambda: ln_finish(s1)])
                sc0 = ln_stats(0)
                h2_0, s2 = B(1, h1, mid_a=[lambda: ln_finish(sc0),
                                           lambda: ln_stats(2)])
                (g2,) = Dp(0, h2_0, mid=[lambda: ln_finish(s2)])
                sc1 = ln_stats(1)
                h2_1, s3 = B(2, g2, mid_a=[lambda: ln_finish(sc1),
                                           lambda: ln_stats(3)])
                (g3,) = Dp(1, h2_1, mid=[lambda: ln_finish(s3)])
                sc2 = ln_stats(2)
                (h2_2,) = B(3, g3, mid_a=[lambda: ln_finish(sc2)])
                last = (l == n_layers - 1)
                if not last:
                    sc3, s0n = Dp(2, h2_2, mid=[lambda: ln_stats(3),
                                                lambda: ln_stats(0)])
                    # note: ln_stats(0) here reads x pair0 as updated by D0 above
                    h2_3 = ln_finish(sc3)
                    (h0n,) = Dp(3, h2_3, mid=[lambda: ln_finish(s0n)])
                    carry = {"h0": h0n, "s1": ln_stats(1)}
                else:
                    (sc3,) = Dp(2, h2_2, mid=[lambda: ln_stats(3)])
                    h2_3 = ln_finish(sc3)
                    Dp(3, h2_3)
            else:
                # simple order for small test configs
                hq = {}
                hq[0] = ln_finish(ln_stats(0))
                for p in range(1, npair):
                    hq[p] = ln_finish(ln_stats(p))
                    B(p - 1, hq.pop(p - 1))
                B(npair - 1, hq.pop(npair - 1))
                hq[0] = ln_finish(ln_stats(0))
                for p in range(1, npair):
                    hq[p] = ln_finish(ln_stats(p))
                    Dp(p - 1, hq.pop(p - 1))
                Dp(npair - 1, hq.pop(npair - 1))

        # ---------------- final LN + head ----------------
        for p in range(npair):
            hf = ln_finish(ln_stats(p))
            for bi in range(PW // T):
                b = p * (PW // T) + bi
                hps = ppv.tile([V, 256], F32, tag="ppv")
                for kc in range(KC):
                    mm(hps[:, :], whead[:, kc, :], hf[:, kc, bi * T:(bi + 1) * T],
                       first=(kc == 0), last=(kc == KC - 1))
                lg = lgpool.tile([V, T], F32, tag="lg")
                nc.vector.tensor_scalar(lg[:, :], hps[:, :], bhead[0:V, 0:1], None,
                                        op0=OP.add)
                nc.sync.dma_start(out=out_d[b], in_=lg[:, :])

    nc.compile()
    return nc


# ---------------------------------------------------------------------------
# host side
# ---------------------------------------------------------------------------

def prep_inputs(inputs, n_layers=L, n_b=BL, core=0):
    """Build the per-core input map (numpy) for `core`."""
    f32 = np.float32
    idx = np.asarray(inputs["idx"])
    tok_emb = np.asarray(inputs["tok_emb"], f32)
    pos_emb = np.asarray(inputs["pos_emb"], f32)
    Wq = np.asarray(inputs["Wq"], f32)
    Wk = np.asarray(inputs["Wk"], f32)
    Wv = np.asarray(inputs["Wv"], f32)
    Wproj = np.asarray(inputs["Wproj"], f32)
    bproj = np.asarray(inputs["bproj"], f32)
    W1 = np.asarray(inputs["W1"], f32)
    b1 = np.asarray(inputs["b1"], f32)
    W2 = np.asarray(inputs["W2"], f32)
    b2 = np.asarray(inputs["b2"], f32)
    ln1_g = np.asarray(inputs["ln1_g"], f32)
    ln1_b = np.asarray(inputs["ln1_b"], f32)
    ln2_g = np.asarray(inputs["ln2_g"], f32)
    ln2_b = np.asarray(inputs["ln2_b"], f32)
    lnf_g = np.asarray(inputs["lnf_g"], f32)
    lnf_b = np.asarray(inputs["lnf_b"], f32)
    Whead = np.asarray(inputs["Whead"], f32)
    bhead = np.asarray(inputs["bhead"], f32)

    ntok = n_b * T
    scale = f32(D) ** -0.5

    idx_c = idx[core * n_b:(core + 1) * n_b].reshape(-1)         # [ntok]
    hot = (idx_c[None, :] == np.arange(V)[:, None]).astype(f32)  # [V, ntok]

    posT = pos_emb.T.astype(f32)                                 # [D, T]
    posT2 = np.concatenate([posT, posT], axis=1)                 # [D, 512]
    pos_in = posT2.reshape(KC, 128, 512).transpose(1, 0, 2).copy()

    lane = np.arange(128)
    t = np.arange(T)
    m0 = (lane[:, None] <= t[None, :]).astype(f32)
    m1 = ((lane[:, None] + 128) <= t[None, :]).astype(f32)
    mask = np.concatenate([m0, m1], axis=1)                      # [128, 512]

    def pack_w(w):  # [D_in, N] -> [128, KC_in, N]
        kin = w.shape[0] // 128
        return w.reshape(kin, 128, -1).transpose(1, 0, 2).copy()

    wqkv = np.zeros((n_layers, 128, 3, KC, D), f32)
    wproj = np.zeros((n_layers, 128, KC, D), f32)
    w1 = np.zeros((n_layers, 128, KC, DFF), f32)
    w2 = np.zeros((n_layers, 128, FT, D), f32)
    vbias = np.zeros((n_layers, 128, D), f32)
    biasc = np.zeros((n_layers, 128, MT + FT + MT), f32)
    biasr = np.zeros((n_layers, 1, 2 * D), f32)

    for l in range(n_layers):
        # Wq[l] is [H, D, HD]; feature f = h*HD+hd -> transpose to [D, H, HD]
        wq2 = Wq[l].transpose(1, 0, 2).reshape(D, D) * scale
        wk2 = Wk[l].transpose(1, 0, 2).reshape(D, D)
        wv2 = Wv[l].transpose(1, 0, 2).reshape(D, D)
        wqkv[l, :, 0] = pack_w(wq2 * ln1_g[l][:, None])
        wqkv[l, :, 1] = pack_w(wk2 * ln1_g[l][:, None])
        wqkv[l, :, 2] = pack_w(wv2 * ln1_g[l][:, None])
        biasr[l, 0, 0:D] = ln1_b[l] @ wq2
        biasr[l, 0, D:2 * D] = ln1_b[l] @ wk2
        vbias[l] = np.broadcast_to(ln1_b[l] @ wv2, (128, D))
        wproj[l] = pack_w(Wproj[l])
        w1[l] = pack_w(W1[l] * ln2_g[l][:, None])
        w2[l] = pack_w(W2[l])
        biasc[l, :, 0:MT] = bproj[l].reshape(MT, 128).T
        biasc[l, :, MT:MT + FT] = (b1[l] + ln2_b[l] @ W1[l]).reshape(FT, 128).T
        biasc[l, :, MT + FT:] = b2[l].reshape(MT, 128).T

    whead_eff = Whead * lnf_g[:, None]
    bhead_eff = (bhead + lnf_b @ Whead).astype(f32)

    cst = np.ones((128, CST_W), f32)
    cst[:, 512] = 1.0 / D
    cst[:, 513:641] = np.eye(128, dtype=f32)
    cst[:, 641:769] = 1.0 / D

    return {
        "cst": cst,
        "hotT": hot,
        "temb": tok_emb.astype(f32),
        "posT": pos_in,
        "mask": mask,
        "wqkv": wqkv,
        "wproj": wproj,
        "w1": w1,
        "w2": w2,
        "vbias": vbias,
        "biasc": biasc,
        "biasr": biasr,
        "whead": pack_w(whead_eff),
        "bheadc": bhead_eff[:, None].copy(),
    }


_CACHE = {}


def get_program():
    if "nc" not in _CACHE:
        _CACHE["nc"] = build_program()
    return _CACHE["nc"]


def run_on_hw(inputs, trace=False):
    nc = get_program()
    in_maps = [prep_inputs(inputs, core=c) for c in range(NCORES)]
    res = run_bass_kernel_spmd(nc, in_maps, list(range(NCORES)), trace=trace)
    outs = []
    for c in range(NCORES):
        lt = res.results[c]["logitsT"]          # [BL, V, T]
        outs.append(lt.transpose(0, 2, 1))      # [BL, T, V]
    full = np.concatenate(outs, axis=0)         # [B, T, V]
    return full, res


def kernel(**inputs):
    out, _ = run_on_hw(inputs, trace=False)
    return out



# revision 6
# speedup vs baseline: 1.3788x; 1.3788x over previous
"""Trainium2 Bass kernel: GPT-style transformer forward pass.

Strategy: data-parallel over batch across 8 NeuronCores (B=64 -> 8 per core),
weights replicated.  All matmuls/activations in bf16 (PE bf16 = 1 cyc/row at
any moving-dim size, DVE 2x on 16-bit, half DMA/SBUF), PSUM accumulation f32.

Mean-free residual trick: tok/pos embeddings and the output columns of
Wproj/W2 (+ bproj/b2) are centered over the feature dim host-side, so the
residual stream x stays exactly zero-mean and LayerNorm needs NO mean
subtraction (logits are invariant: every consumer of x is a LayerNorm).
LN reduces to rstd = (E[x^2]+eps)^-1/2 via one ones-matmul of x^2 and
h = x * rstd.  Q/K biases are applied on the PSUM->SBUF copy (per-partition
bias columns) instead of via rank-1 matmuls.

Attention: transposed scores S_T[s,t] per (b,h), multiplicative causal mask
after exp, token-major PV in bf16 (moving dim 66 at full bf16 rate) with an
appended ones-column in V so softmax denominators land as a per-partition
column, and bf16 PE transposes carry att back to feature-major.
"""

import os
import sys

for _p in ("/opt/trn_rl_repo",):
    if _p not in sys.path and os.path.isdir(_p):
        sys.path.insert(0, _p)

import numpy as np
import ml_dtypes

import concourse.bass as bass
import concourse.mybir as mybir
import concourse.tile as tile
from concourse import bacc
from concourse.bass_utils import run_bass_kernel_spmd

F32 = mybir.dt.float32
BF16 = mybir.dt.bfloat16
AF = mybir.ActivationFunctionType
OP = mybir.AluOpType

NPBF = ml_dtypes.bfloat16

V, D, H, HD, L, T, B = 65, 384, 6, 64, 6, 256, 64
NCORES = 8
BL = B // NCORES          # batch elements per core
NTOK = BL * T             # tokens per core
DFF = 4 * D               # 1536
EPS = 1e-3
KC = D // 128             # 3 contraction chunks of 128
MT = D // 128             # 3 output feature tiles
FT = DFF // 128           # 12 ffn tiles
HD1 = HD + 2              # V columns incl ones col (64) + pad (65)
MDT = BF16

# stat matmul uses 1/512 (exact in bf16); corrections folded into Ln/Exp:
#   msq = sum(x^2)/512 = var * D/512
#   rstd = exp(-0.5*ln(msq + EPS*D/512) + 0.5*ln(D/512))
EPS_EFF = EPS * D / 512.0
RSTD_BIAS = 0.5 * float(np.log(D / 512.0))

# cst layout: cols 0:512 ones, col 512 unused, cols 513:641 identity 128x128,
# cols 641:769 = 1/512 block (stats matmul lhsT)
CST_W = 513 + 128 + 128


def _patch_act_tables():
    """Steer the activation-table picker to natural_log_exp_and_others for
    Exp and Ln, so this kernel's ACT stream never switches table sets."""
    if getattr(bacc, "_act_tables_patched", False):
        return
    real = bacc.get_activation_tables

    def patched(arch):
        t = real(arch)
        exp = mybir.ActivationFunctionType.Exp
        ln = mybir.ActivationFunctionType.Ln
        out = {}
        for name, fns in t.items():
            if name != "natural_log_exp_and_others":
                fns = fns - {exp, ln}
            out[name] = fns
        return out

    bacc.get_activation_tables = patched
    bacc._act_tables_patched = True


class _MM:
    """matmul emitter with explicit accumulation-chain boundaries."""

    def __init__(self, nc):
        self.nc = nc

    def __call__(self, out, lhsT, rhs, first=True, last=True, tile_position=None):
        self.nc.tensor.matmul(
            out, lhsT, rhs, start=first, stop=last, tile_position=tile_position,
        )


def build_program(n_layers=L, n_b=BL, n_heads=H):
    _patch_act_tables()
    assert n_b % 2 == 0 or n_b == 1
    ntok = n_b * T
    npair = max(1, n_b // 2)
    PW = 512 if n_b > 1 else 256      # tokens per pair-chunk
    nc = bacc.Bacc("TRN2", target_bir_lowering=False, debug=False)

    # ---------------- DRAM I/O ----------------
    hot_d = nc.dram_tensor("hotT", [V, ntok], MDT, kind="ExternalInput").ap()
    temb_d = nc.dram_tensor("temb", [V, D], MDT, kind="ExternalInput").ap()
    pos_d = nc.dram_tensor("posT", [128, KC, 512], F32, kind="ExternalInput").ap()
    mask_d = nc.dram_tensor("mask", [128, 512], MDT, kind="ExternalInput").ap()
    wqkv_d = nc.dram_tensor("wqkv", [n_layers, 128, 3, KC, D], MDT, kind="ExternalInput").ap()
    wproj_d = nc.dram_tensor("wproj", [n_layers, 128, KC, D], MDT, kind="ExternalInput").ap()
    w1_d = nc.dram_tensor("w1", [n_layers, 128, KC, DFF], MDT, kind="ExternalInput").ap()
    w2_d = nc.dram_tensor("w2", [n_layers, 128, FT, D], MDT, kind="ExternalInput").ap()
    vbias_d = nc.dram_tensor("vbias", [n_layers, 128, D], F32, kind="ExternalInput").ap()
    biasc_d = nc.dram_tensor("biasc", [n_layers, 128, MT + FT + MT], F32, kind="ExternalInput").ap()
    biasr_d = nc.dram_tensor("biasr", [n_layers, 128, 2 * MT], F32, kind="ExternalInput").ap()
    whead_d = nc.dram_tensor("whead", [128, KC, V], MDT, kind="ExternalInput").ap()
    bhead_d = nc.dram_tensor("bheadc", [V, 1], F32, kind="ExternalInput").ap()
    lnc_d = nc.dram_tensor("lnc", [128, 2], F32, kind="ExternalInput").ap()
    cst_d = nc.dram_tensor("cst", [128, CST_W], MDT, kind="ExternalInput").ap()
    out_d = nc.dram_tensor("logitsT", [n_b, V, T], F32, kind="ExternalOutput").ap()

    from contextlib import ExitStack

    with tile.TileContext(nc) as tc, \
         nc.allow_low_precision(reason="bf16 matmul operand production"), \
         ExitStack() as ctx:
        ep = ctx.enter_context

        # ---------------- pools ----------------
        cpool = ep(tc.tile_pool(name="consts", bufs=1))
        xpool = ep(tc.tile_pool(name="x", bufs=1))
        wpool_qkv = ep(tc.tile_pool(name="wqkv", bufs=1))
        wpool_proj = ep(tc.tile_pool(name="wproj", bufs=1))
        wpool_1 = ep(tc.tile_pool(name="w1", bufs=1))
        wpool_2 = ep(tc.tile_pool(name="w2", bufs=1))
        wpool_b = ep(tc.tile_pool(name="wbias", bufs=1))
        hpool = ep(tc.tile_pool(name="h", bufs=3))
        xsqpool = ep(tc.tile_pool(name="xsq", bufs=1))
        qpool = ep(tc.tile_pool(name="q", bufs=2))
        kpool = ep(tc.tile_pool(name="k", bufs=2))
        vpool = ep(tc.tile_pool(name="v", bufs=1))
        upool = ep(tc.tile_pool(name="u", bufs=4))
        atmpool = ep(tc.tile_pool(name="atm", bufs=2))
        attpool = ep(tc.tile_pool(name="att", bufs=1))
        h1pool = ep(tc.tile_pool(name="h1", bufs=1))
        lgpool = ep(tc.tile_pool(name="lg", bufs=1))
        stpool = ep(tc.tile_pool(name="st", bufs=6))
        rdpool = ep(tc.tile_pool(name="rd", bufs=4))

        pbig = ep(tc.tile_pool(name="pbig", bufs=4, space="PSUM"))
        ppv = ep(tc.tile_pool(name="ppv", bufs=2, space="PSUM"))
        pstat = ep(tc.tile_pool(name="pstat", bufs=2, space="PSUM"))

        mm = _MM(nc)

        # ---------------- constants ----------------
        cst = cpool.tile([128, CST_W], MDT, name="cst_c")
        nc.sync.dma_start(out=cst[:, :], in_=cst_d[:, :])
        ones = cst[:, 0:512]
        ident = cst[:, 513:641]
        inv512 = cst[:, 641:769]
        mask = cpool.tile([128, 512], MDT, name="mask_c")
        nc.sync.dma_start(out=mask[:, :], in_=mask_d[:, :])
        whead = cpool.tile([128, KC, V], MDT, name="whead_c")
        nc.sync.dma_start(out=whead[:, :, :], in_=whead_d[:, :, :])
        bhead = cpool.tile([V, 1], F32, name="bhead_c")
        nc.sync.dma_start(out=bhead[:, :], in_=bhead_d[:, :])
        lnc = cpool.tile([128, 2], F32, name="lnc_c")
        nc.sync.dma_start(out=lnc[:, :], in_=lnc_d[:, :])

        x = xpool.tile([128, KC, ntok], MDT, name="x_resid")

        nbp = PW // T
        v_tiles = [vpool.tile([128, 2 * nbp, n_heads, HD1], MDT, name=f"v_pp{i}")
                   for i in range(2)]
        for vt in v_tiles:
            for tb in range(2 * nbp):
                nc.vector.tensor_copy(vt[:, tb, :, HD:HD1],
                                      ones[:, 0:2 * n_heads].rearrange(
                                          "p (h c) -> p h c", h=n_heads))

        # ---------------- embedding (scratch tiles borrow other pools) ----------------
        pos = attpool.tile([128, KC, 512], F32, tag="att")
        nc.sync.dma_start(out=pos[:, :, :], in_=pos_d[:, :, :])
        temb = stpool.tile([V, 384], MDT, tag="st")
        nc.sync.dma_start(out=temb[:, :], in_=temb_d[:, :])
        for ch in range(ntok // PW):
            cs = slice(ch * PW, ch * PW + PW)
            hot = xsqpool.tile([V, PW], MDT, tag="xsq")
            nc.sync.dma_start(out=hot[:, :], in_=hot_d[:, cs])
            for c in range(KC):
                ps = pbig.tile([128, 512], F32, tag="pbig")
                mm(ps[:, 0:PW], temb[0:V, c * 128:(c + 1) * 128], hot[0:V, :])
                nc.vector.tensor_tensor(x[:, c, cs], ps[:, 0:PW], pos[:, c, 0:PW], op=OP.add)

        # ---------------- LN (mean-free) split into stats + finish ----------------
        def ln_stats(p):
            """x^2 -> ones-matmul partition reduction -> rstd, all off DVE"""
            pc = slice(p * PW, p * PW + PW)
            xsq = xsqpool.tile([128, KC, PW], MDT, tag="xsq")
            nc.scalar.activation(xsq[:, :, :], x[:, :, pc], AF.Square)
            msqb = pstat.tile([128, PW], F32, tag="pstat")
            for c in range(KC):
                mm(msqb[:, :], inv512[:, :], xsq[:, c, :],
                   first=(c == 0), last=(c == KC - 1))
            lnv = stpool.tile([128, PW], F32, tag="st")
            nc.scalar.activation(lnv[:, :], msqb[:, :], AF.Ln, bias=lnc[:, 0:1])
            rstdb = stpool.tile([128, PW], MDT, tag="st")
            nc.scalar.activation(rstdb[:, :], lnv[:, :], AF.Exp,
                                 scale=-0.5, bias=lnc[:, 1:2])
            return (p, rstdb)

        def ln_finish(tok):
            """apply -> h (pure DVE; no PE work)"""
            p, rstdb = tok
            pc = slice(p * PW, p * PW + PW)
            h = hpool.tile([128, KC, PW], MDT, tag="h")
            for c in range(KC):
                nc.vector.tensor_tensor(h[:, c, :], x[:, c, pc], rstdb[:, :], op=OP.mult)
            return h

        def run(mids):
            out = []
            for f in mids:
                out.append(f())
            return out

        # ---------------- phase B: qkv + attention + proj for a pair ----------------
        def emit_B(p, h, wqkv, wproj, vbias, biasc, biasr, mid_a=(), mid_b=()):
            pc = slice(p * PW, p * PW + PW)
            nb_in_p = PW // T
            q_t = qpool.tile([128, MT, PW], MDT, tag="q")
            k_t = kpool.tile([128, MT, PW], MDT, tag="k")
            for mat, dst in ((0, q_t), (1, k_t)):
                for mt in range(MT):
                    ps = pbig.tile([128, 512], F32, tag="pbig")
                    for kc in range(KC):
                        mm(ps[:, 0:PW], wqkv[:, mat, kc, mt * 128:(mt + 1) * 128],
                           h[:, kc, :], first=(kc == 0), last=(kc == KC - 1))
                    nc.vector.tensor_scalar(
                        dst[:, mt, :], ps[:, 0:PW],
                        biasr[:, mat * MT + mt:mat * MT + mt + 1], None, op0=OP.add)
            v_t = v_tiles[p % 2]
            for tb in range(2 * nb_in_p):
                vps = pbig.tile([128, 512], F32, tag="pbig")
                for kc in range(KC):
                    mm(vps[:, 0:D], h[:, kc, tb * 128:(tb + 1) * 128],
                       wqkv[:, 2, kc, :], first=(kc == 0), last=(kc == KC - 1))
                nc.vector.tensor_tensor(
                    v_t[:, tb, :, 0:HD],
                    vps[:, 0:D].rearrange("p (h d) -> p h d", h=n_heads),
                    vbias[:, :].rearrange("p (h d) -> p h d", h=n_heads),
                    op=OP.add)
            mids_out = run(mid_a)
            atms = []
            for bi in range(nb_in_p):
                boff = bi * T
                if bi == 1:
                    mids_out += run(mid_b)
                us = [None] * n_heads
                pv0 = ppv.tile([128, n_heads, HD1], F32, tag="ppv")
                pv1 = ppv.tile([128, n_heads, HD1], F32, tag="ppv")

                def emit_S(hh):
                    hp = 64 * (hh % 2)
                    hc = hh // 2
                    sps = pbig.tile([128, 512], F32, tag="pbig")
                    mm(sps[:, 0:256], k_t[hp:hp + HD, hc, boff:boff + 128],
                       q_t[hp:hp + HD, hc, boff:boff + T])
                    mm(sps[:, 256:512], k_t[hp:hp + HD, hc, boff + 128:boff + 256],
                       q_t[hp:hp + HD, hc, boff:boff + T])
                    u_t = upool.tile([128, 512], MDT, tag="u")
                    nc.scalar.activation(u_t[:, :], sps[:, :], AF.Exp)
                    nc.vector.tensor_tensor(u_t[:, :], u_t[:, :], mask[:, :], op=OP.mult)
                    us[hh] = u_t

                def emit_PV(hh):
                    u_t = us[hh]
                    mm(pv0[:, hh, :], u_t[:, 0:128], v_t[:, 2 * bi, hh, :])
                    mm(pv1[:, hh, :], u_t[:, 128:256], v_t[:, 2 * bi, hh, :],
                       first=True, last=False)
                    mm(pv1[:, hh, :], u_t[:, 384:512], v_t[:, 2 * bi + 1, hh, :],
                       first=False, last=True)

                # window the S/PV interleave so only ~3 U tiles are live
                emit_S(0); emit_S(1); emit_S(2)
                emit_PV(0); emit_S(3)
                emit_PV(1); emit_S(4)
                emit_PV(2); emit_S(5)
                emit_PV(3); emit_PV(4); emit_PV(5)
                atm = atmpool.tile([128, 2, n_heads * HD], MDT, tag="atm")
                for tb, pv in ((0, pv0), (1, pv1)):
                    rden = rdpool.tile([128, n_heads], F32, tag="rd")
                    nc.vector.reciprocal(rden[:, :], pv[:, :, HD])
                    nc.vector.tensor_tensor(
                        atm[:, tb, :].rearrange("p (h d) -> p h d", h=n_heads),
                        pv[:, :, 0:HD],
                        rden[:, :, None].broadcast_to([128, n_heads, HD]),
                        op=OP.mult)
                atms.append(atm)
            att_t = attpool.tile([128, KC, PW], MDT, tag="att")
            for c in range(KC):
                tps = pbig.tile([128, 512], MDT, tag="pbig")
                for bi in range(nb_in_p):
                    for tb in range(2):
                        col = (bi * 2 + tb) * 128
                        nc.tensor.transpose(
                            tps[:, col:col + 128],
                            atms[bi][:, tb, c * 128:(c + 1) * 128],
                            ident[:, :])
                nc.vector.tensor_copy(att_t[:, c, :], tps[:, 0:PW])
            for mt in range(MT):
                pp = pbig.tile([128, 512], F32, tag="pbig")
                for kc in range(KC):
                    mm(pp[:, 0:PW], wproj[:, kc, mt * 128:(mt + 1) * 128],
                       att_t[:, kc, :], first=(kc == 0), last=(kc == KC - 1))
                nc.vector.scalar_tensor_tensor(
                    x[:, mt, pc], pp[:, 0:PW], biasc[:, mt:mt + 1], x[:, mt, pc],
                    op0=OP.add, op1=OP.add)
            return mids_out

        # ---------------- phase D: FFN for a pair ----------------
        def emit_D(p, h2, w1, w2, biasc, mid=()):
            pc = slice(p * PW, p * PW + PW)
            h1_t = h1pool.tile([128, FT, PW], MDT, tag="h1")
            for mt in range(FT):
                fps = pbig.tile([128, 512], F32, tag="pbig")
                for kc in range(KC):
                    mm(fps[:, 0:PW], w1[:, kc, mt * 128:(mt + 1) * 128],
                       h2[:, kc, :], first=(kc == 0), last=(kc == KC - 1))
                nc.scalar.activation(h1_t[:, mt, :], fps[:, 0:PW], AF.Relu,
                                     bias=biasc[:, MT + mt:MT + mt + 1])
            mids_out = run(mid)
            for mt in range(MT):
                fp2 = pbig.tile([128, 512], F32, tag="pbig")
                for kc in range(FT):
                    mm(fp2[:, 0:PW], w2[:, kc, mt * 128:(mt + 1) * 128],
                       h1_t[:, kc, :], first=(kc == 0), last=(kc == FT - 1))
                nc.vector.scalar_tensor_tensor(
                    x[:, mt, pc], fp2[:, 0:PW],
                    biasc[:, MT + FT + mt:MT + FT + mt + 1],
                    x[:, mt, pc], op0=OP.add, op1=OP.add)
            return mids_out

        # ---------------- layers: software-pipelined emission ----------------
        carry = {}
        for l in range(n_layers):
            wqkv = wpool_qkv.tile([128, 3, KC, D], MDT, tag="wqkv")
            nc.sync.dma_start(out=wqkv[:, :, :, :], in_=wqkv_d[l])
            wproj = wpool_proj.tile([128, KC, D], MDT, tag="wproj")
            nc.sync.dma_start(out=wproj[:, :, :], in_=wproj_d[l])
            w1 = wpool_1.tile([128, KC, DFF], MDT, tag="w1")
            nc.sync.dma_start(out=w1[:, :, :], in_=w1_d[l])
            w2 = wpool_2.tile([128, FT, D], MDT, tag="w2")
            nc.sync.dma_start(out=w2[:, :, :], in_=w2_d[l])
            vbias = wpool_b.tile([128, D], F32, tag="vbias")
            nc.sync.dma_start(out=vbias[:, :], in_=vbias_d[l])
            biasc = wpool_b.tile([128, MT + FT + MT], F32, tag="biasc")
            nc.sync.dma_start(out=biasc[:, :], in_=biasc_d[l])
            biasr = wpool_b.tile([128, 2 * MT], F32, tag="biasr")
            nc.sync.dma_start(out=biasr[:, :], in_=biasr_d[l])

            Bf = lambda p, h, **kw: emit_B(p, h, wqkv, wproj, vbias, biasc, biasr, **kw)
            Dp = lambda p, h2, **kw: emit_D(p, h2, w1, w2, biasc, **kw)

            if npair == 4:
                if l == 0:
                    h0 = ln_finish(ln_stats(0))
                    s1 = ln_stats(1)
                else:
                    h0, s1 = carry["h0"], carry["s1"]
                (h1,) = Bf(0, h0, mid_a=[lambda: ln_finish(s1)])
                sc0 = ln_stats(0)
                h2_0, s2 = Bf(1, h1, mid_a=[lambda: ln_finish(sc0),
                                            lambda: ln_stats(2)])
                (g2,) = Dp(0, h2_0, mid=[lambda: ln_finish(s2)])
                sc1 = ln_stats(1)
                h2_1, s3 = Bf(2, g2, mid_a=[lambda: ln_finish(sc1),
                                            lambda: ln_stats(3)])
                (g3,) = Dp(1, h2_1, mid=[lambda: ln_finish(s3)])
                sc2 = ln_stats(2)
                (h2_2,) = Bf(3, g3, mid_a=[lambda: ln_finish(sc2)])
                last = (l == n_layers - 1)
                if not last:
                    sc3, s0n = Dp(2, h2_2, mid=[lambda: ln_stats(3),
                                                lambda: ln_stats(0)])
                    # note: ln_stats(0) here reads x pair0 as updated by D0 above
                    h2_3 = ln_finish(sc3)
                    (h0n,) = Dp(3, h2_3, mid=[lambda: ln_finish(s0n)])
                    carry = {"h0": h0n, "s1": ln_stats(1)}
                else:
                    (sc3,) = Dp(2, h2_2, mid=[lambda: ln_stats(3)])
                    h2_3 = ln_finish(sc3)
                    Dp(3, h2_3)
            else:
                # simple order for small test configs
                hq = {}
                hq[0] = ln_finish(ln_stats(0))
                for p in range(1, npair):
                    hq[p] = ln_finish(ln_stats(p))
                    Bf(p - 1, hq.pop(p - 1))
                Bf(npair - 1, hq.pop(npair - 1))
                hq[0] = ln_finish(ln_stats(0))
                for p in range(1, npair):
                    hq[p] = ln_finish(ln_stats(p))
                    Dp(p - 1, hq.pop(p - 1))
                Dp(npair - 1, hq.pop(npair - 1))

        # ---------------- final LN + head ----------------
        for p in range(npair):
            hf = ln_finish(ln_stats(p))
            for bi in range(PW // T):
                b = p * (PW // T) + bi
                hps = ppv.tile([V, 256], F32, tag="ppv")
                for kc in range(KC):
                    mm(hps[:, :], whead[:, kc, :], hf[:, kc, bi * T:(bi + 1) * T],
                       first=(kc == 0), last=(kc == KC - 1))
                lg = lgpool.tile([V, T], F32, tag="lg")
                nc.vector.tensor_scalar(lg[:, :], hps[:, :], bhead[0:V, 0:1], None,
                                        op0=OP.add)
                nc.sync.dma_start(out=out_d[b], in_=lg[:, :])

    nc.compile()
    return nc


# ---------------------------------------------------------------------------
# host side
# ---------------------------------------------------------------------------

def prep_inputs(inputs, n_layers=L, n_b=BL, core=0):
    """Build the per-core input map (numpy) for `core`."""
    f32 = np.float32
    idx = np.asarray(inputs["idx"])
    tok_emb = np.asarray(inputs["tok_emb"], f32)
    pos_emb = np.asarray(inputs["pos_emb"], f32)
    Wq = np.asarray(inputs["Wq"], f32)
    Wk = np.asarray(inputs["Wk"], f32)
    Wv = np.asarray(inputs["Wv"], f32)
    Wproj = np.asarray(inputs["Wproj"], f32)
    bproj = np.asarray(inputs["bproj"], f32)
    W1 = np.asarray(inputs["W1"], f32)
    b1 = np.asarray(inputs["b1"], f32)
    W2 = np.asarray(inputs["W2"], f32)
    b2 = np.asarray(inputs["b2"], f32)
    ln1_g = np.asarray(inputs["ln1_g"], f32)
    ln1_b = np.asarray(inputs["ln1_b"], f32)
    ln2_g = np.asarray(inputs["ln2_g"], f32)
    ln2_b = np.asarray(inputs["ln2_b"], f32)
    lnf_g = np.asarray(inputs["lnf_g"], f32)
    lnf_b = np.asarray(inputs["lnf_b"], f32)
    Whead = np.asarray(inputs["Whead"], f32)
    bhead = np.asarray(inputs["bhead"], f32)

    ntok = n_b * T
    scale = f32(D) ** -0.5

    # mean-free residual: center embeddings + Wproj/W2 output columns
    tok_emb = tok_emb - tok_emb.mean(-1, keepdims=True)
    pos_emb = pos_emb - pos_emb.mean(-1, keepdims=True)
    Wproj = Wproj - Wproj.mean(-1, keepdims=True)
    W2 = W2 - W2.mean(-1, keepdims=True)
    bproj = bproj - bproj.mean(-1, keepdims=True)
    b2 = b2 - b2.mean(-1, keepdims=True)

    idx_c = idx[core * n_b:(core + 1) * n_b].reshape(-1)         # [ntok]
    hot = (idx_c[None, :] == np.arange(V)[:, None]).astype(NPBF)  # [V, ntok]

    posT = pos_emb.T.astype(f32)                                 # [D, T]
    posT2 = np.concatenate([posT, posT], axis=1)                 # [D, 512]
    pos_in = posT2.reshape(KC, 128, 512).transpose(1, 0, 2).copy()

    lane = np.arange(128)
    t = np.arange(T)
    m0 = (lane[:, None] <= t[None, :]).astype(NPBF)
    m1 = ((lane[:, None] + 128) <= t[None, :]).astype(NPBF)
    mask = np.concatenate([m0, m1], axis=1)                      # [128, 512]

    def pack_w(w):  # [D_in, N] -> [128, KC_in, N]
        kin = w.shape[0] // 128
        return w.reshape(kin, 128, -1).transpose(1, 0, 2).copy()

    wqkv = np.zeros((n_layers, 128, 3, KC, D), NPBF)
    wproj = np.zeros((n_layers, 128, KC, D), NPBF)
    w1 = np.zeros((n_layers, 128, KC, DFF), NPBF)
    w2 = np.zeros((n_layers, 128, FT, D), NPBF)
    vbias = np.zeros((n_layers, 128, D), f32)
    biasc = np.zeros((n_layers, 128, MT + FT + MT), f32)
    biasr = np.zeros((n_layers, 128, 2 * MT), f32)

    for l in range(n_layers):
        # Wq[l] is [H, D, HD]; feature f = h*HD+hd -> transpose to [D, H, HD]
        wq2 = Wq[l].transpose(1, 0, 2).reshape(D, D) * scale
        wk2 = Wk[l].transpose(1, 0, 2).reshape(D, D)
        wv2 = Wv[l].transpose(1, 0, 2).reshape(D, D)
        wqkv[l, :, 0] = pack_w(wq2 * ln1_g[l][:, None])
        wqkv[l, :, 1] = pack_w(wk2 * ln1_g[l][:, None])
        wqkv[l, :, 2] = pack_w(wv2 * ln1_g[l][:, None])
        biasr[l, :, 0:MT] = (ln1_b[l] @ wq2).reshape(MT, 128).T
        biasr[l, :, MT:2 * MT] = (ln1_b[l] @ wk2).reshape(MT, 128).T
        vbias[l] = np.broadcast_to(ln1_b[l] @ wv2, (128, D))
        wproj[l] = pack_w(Wproj[l])
        w1[l] = pack_w(W1[l] * ln2_g[l][:, None])
        w2[l] = pack_w(W2[l])
        biasc[l, :, 0:MT] = bproj[l].reshape(MT, 128).T
        biasc[l, :, MT:MT + FT] = (b1[l] + ln2_b[l] @ W1[l]).reshape(FT, 128).T
        biasc[l, :, MT + FT:] = b2[l].reshape(MT, 128).T

    whead_eff = Whead * lnf_g[:, None]
    bhead_eff = (bhead + lnf_b @ Whead).astype(f32)

    cst = np.ones((128, CST_W), NPBF)
    cst[:, 513:641] = np.eye(128, dtype=NPBF)
    cst[:, 641:769] = NPBF(1.0 / 512.0)

    lnc = np.zeros((128, 2), f32)
    lnc[:, 0] = EPS_EFF
    lnc[:, 1] = RSTD_BIAS

    return {
        "cst": cst,
        "lnc": lnc,
        "hotT": hot,
        "temb": tok_emb.astype(NPBF),
        "posT": pos_in,
        "mask": mask,
        "wqkv": wqkv,
        "wproj": wproj,
        "w1": w1,
        "w2": w2,
        "vbias": vbias,
        "biasc": biasc,
        "biasr": biasr,
        "whead": pack_w(whead_eff).astype(NPBF),
        "bheadc": bhead_eff[:, None].copy(),
    }


_CACHE = {}


def get_program():
    if "nc" not in _CACHE:
        _CACHE["nc"] = build_program()
    return _CACHE["nc"]


def run_on_hw(inputs, trace=False):
    nc = get_program()
    in_maps = [prep_inputs(inputs, core=c) for c in range(NCORES)]
    res = run_bass_kernel_spmd(nc, in_maps, list(range(NCORES)), trace=trace)
    outs = []
    for c in range(NCORES):
        lt = res.results[c]["logitsT"]          # [BL, V, T]
        outs.append(lt.transpose(0, 2, 1))      # [BL, T, V]
    full = np.concatenate(outs, axis=0)         # [B, T, V]
    return full, res


def kernel(**inputs):
    out, _ = run_on_hw(inputs, trace=False)
    return out
